# revision 45
# baseline (speedup 1.0000x reference)
"""Trainium2 Bass kernel for a dense causal self-attention block (RoPE + causal
softmax + QKV/O projections).

Sharding: 8 cores = 2 batches x 4 head-groups (tensor parallel over heads).
Each core computes 4 heads of attention for one batch plus the partial output
projection over its heads' dims; the host sums the 4 partial outputs per batch.

Causal-mode design (fused single-pass pipeline):
  - All matmul operands are bf16 (same PE rate as fp32r, but full rate at any
    N, half the SBUF/DMA, and q/k/v/et/avt all stay SBUF-resident -- no DRAM
    round trips). Accumulation is fp32 in PSUM; overall rel err ~3e-3.
  - Causality makes attention for q-chunk qc depend only on K/V of chunks
    <= qc, so QKV projection (A), attention (B) and O-projection (C) stream as
    one software-pipelined pass over the 4 q-chunks: window qc runs B(qc) on
    the PE woven with A(qc+1) and C(qc-1) units so the PE never idles (PE
    p-state drops to half clock after any gap, so gaps are doubly costly).
  - Scores are computed transposed (S^T[k,q] = K^T_tile.T @ Q^T) with the
    diagonal blocks trimmed to their live q-columns (bf16 runs full rate below
    N=256, unlike fp32r). The causal mask is a single 0/1 bf16 [128,128]
    triangle multiplied into et AFTER exp -- off the score->exp critical
    chain (AV/den read et LAG steps later, so the DVE mul has slack).
  - The softmax denominator never touches the PE: per-ki partition-sums run on
    the otherwise idle GPSIMD engine (tensor_reduce axis=C), a second-level
    reduce + partition_broadcast produce 1/den, and DVE applies it at the AV^T
    PSUM eviction (deferred one block so the PE stream never waits).
  - Output partials are stored bf16 (halves the output-DMA tail); the host
    upcasts and sums the 4 partials per batch.
"""

import numpy as np

# Problem constants (hardcoded per the harness contract).
B = 2
S = 2048
D = 2048
H = 16
DH = 128
N_CORES = 8
GROUPS = 4          # head-groups (cores per batch)
HPC = H // GROUPS   # heads per core
P = 128             # SBUF partitions
QC = 512            # q/s chunk (f32 PSUM bank = 512 floats)

_CACHE = {}


def _ensure_paths():
    import sys
    for p in ("/opt/trn_rl_repo", "/root/.axon_site/_ro/trn_rl_repo"):
        try:
            import concourse.bass  # noqa: F401
            return
        except Exception:
            if p not in sys.path:
                sys.path.insert(0, p)
    import concourse.bass  # noqa: F401


def build_fused_causal():
    """Fused causal-attention program: one pipelined pass over q-chunks."""
    _ensure_paths()
    import concourse.mybir as mybir
    import concourse.tile as tile
    from concourse import bacc

    import concourse.bass_isa as bass_isa

    f32 = mybir.dt.float32
    bf16 = mybir.dt.bfloat16
    Exp = mybir.ActivationFunctionType.Exp
    ROP_ADD = bass_isa.ReduceOp.add

    E = HPC * P          # per-core projection width (512)
    NDT = D // P         # d (contraction) tiles for projections (16)
    NQC = S // QC        # q chunks (4)
    NKT = S // P         # k tiles (16)
    NOC = D // QC        # output chunks for O projection (4)
    JB = QC // P         # diagonal blocks per q chunk (4)
    HF = P // 2
    XG = 4               # d-tiles per x granule
    LAG = 4              # AV trails exp by LAG ki-steps
    scale = 1.0 / float(np.sqrt(DH))

    nc = bacc.Bacc("TRN2", target_bir_lowering=False, debug=False,
                   num_devices=N_CORES)

    xT = nc.dram_tensor("xT", [D, S], bf16, kind="ExternalInput").ap()
    wqT = nc.dram_tensor("wqT", [D, E], bf16, kind="ExternalInput").ap()
    wkT = nc.dram_tensor("wkT", [D, E], bf16, kind="ExternalInput").ap()
    wvT = nc.dram_tensor("wvT", [D, E], bf16, kind="ExternalInput").ap()
    woT = nc.dram_tensor("woT", [E, D], bf16, kind="ExternalInput").ap()
    cosT = nc.dram_tensor("cosT", [P, S], bf16, kind="ExternalInput").ap()
    sinTs = nc.dram_tensor("sinTs", [P, S], bf16, kind="ExternalInput").ap()
    triD = nc.dram_tensor("triD", [P, P], bf16, kind="ExternalInput").ap()
    y = nc.dram_tensor("y", [S, D], bf16, kind="ExternalOutput").ap()

    xT_t = xT.rearrange("(t p) s -> p t s", p=P)
    wqT_t = wqT.rearrange("(t p) e -> p t e", p=P)
    wkT_t = wkT.rearrange("(t p) e -> p t e", p=P)
    wvT_t = wvT.rearrange("(t p) e -> p t e", p=P)
    woT_t = woT.rearrange("(t p) o -> p t o", p=P)

    def mm(ps, lhsT, rhs, start, stop):
        nc.tensor.matmul(ps, lhsT=lhsT, rhs=rhs, start=start, stop=stop)

    with tile.TileContext(nc) as tc:
      with tc.tile_pool(name="persist", bufs=1) as pp, \
           tc.tile_pool(name="xp", bufs=5) as xp, \
           tc.tile_pool(name="evp", bufs=2) as evp, \
           tc.tile_pool(name="smp", bufs=2) as smp, \
           tc.tile_pool(name="yp", bufs=2) as yp, \
           tc.tile_pool(name="ps_qk", bufs=2, space="PSUM") as ps_qk, \
           tc.tile_pool(name="ps_sc", bufs=2, space="PSUM") as ps_sc, \
           tc.tile_pool(name="ps_av", bufs=2, space="PSUM") as ps_av, \
           tc.tile_pool(name="ps_dn", bufs=1, space="PSUM") as ps_dn, \
           tc.tile_pool(name="ps_y", bufs=1, space="PSUM") as ps_y:

        wq_sb = pp.tile([P, NDT * E], bf16)
        wk_sb = pp.tile([P, NDT * E], bf16)
        wv_sb = pp.tile([P, NDT * E], bf16)
        wo_sb = pp.tile([P, HPC * D], bf16)   # [p=e%128, h*D + o]
        qt_sb = pp.tile([P, HPC * S], bf16)   # [p=dh, h*S + s]
        kt_sb = pp.tile([P, HPC * S], bf16)
        v_sb = pp.tile([P, NKT * E], bf16)    # [p=k%128, ki*E + e]
        avt = pp.tile([P, HPC * S], bf16)     # [p=e(dh of h), h*S + q]
        et = pp.tile([P, NKT * QC], bf16)     # exp(scores^T), per (h,qc)
        cos_sb = pp.tile([P, S], bf16)
        sin_sb = pp.tile([P, S], bf16)
        tri_sb = pp.tile([P, P], bf16)   # 0/1 lower-triangle, applied to et
        ones_f = pp.tile([P, 1], f32)
        ones_bf = pp.tile([P, 1], bf16)
        nc.vector.memset(ones_f, 1.0)
        nc.vector.tensor_copy(ones_bf, ones_f)

        WG = 4

        def load_w(w_sb, w_t, g, t0=0, tn=None, queue=None):
            # weights ride the Activation-engine DMA queue so they stream in
            # parallel with x/table loads on the SP queue
            t0 = g * WG + t0
            tn = tn if tn is not None else WG
            (queue or nc.scalar).dma_start(
                w_sb[:, t0 * E:(t0 + tn) * E].rearrange(
                    "p (t e) -> p t e", t=tn),
                w_t[:, t0:t0 + tn])

        xg_tiles = {}

        def dma_x(qc):
            qs = slice(qc * QC, (qc + 1) * QC)
            for g in range(NDT // XG):
                x_g = xp.tile([P, XG * QC], bf16, tag="x", name=f"xg{qc}_{g}")
                nc.sync.dma_start(
                    x_g.rearrange("p (t s) -> p t s", t=XG),
                    xT_t[:, g * XG:(g + 1) * XG, qs])
                xg_tiles[(qc, g)] = x_g

        def xts(qc, di):
            return xg_tiles[(qc, di // XG)][:, (di % XG) * QC:
                                            (di % XG + 1) * QC]

        # ---- A-stream: QKV projections + RoPE for one q-chunk ----
        def a_units(qc, heads=(0, 1, 2, 3), include_v=True):
            qs = slice(qc * QC, (qc + 1) * QC)
            units = []
            vus = []
            for sl in range(QC // P if include_v else 0):
                si = qc * (QC // P) + sl
                st = {}

                def vu(part, sl=sl, si=si, st=st, qc=qc):
                    if part == 0:
                        st['ps'] = ps_qk.tile([P, E], f32, tag="qk",
                                              name=f"psv{si}")
                    ps = st['ps']
                    for di in range(part * 4, part * 4 + 4):
                        mm(ps, xts(qc, di)[:, sl * P:(sl + 1) * P],
                           wv_sb[:, di * E:(di + 1) * E],
                           start=(di == 0), stop=(di == NDT - 1))
                    if part == 3:
                        nc.scalar.copy(v_sb[:, si * E:(si + 1) * E], ps)
                vus.append(vu)
            # pair sl-streams (2 live psums) so each x granule is consumed
            # twice as fast on first touch -- matters in the DMA-paced prologue
            if include_v:
                for a, b in ((0, 1), (2, 3)):
                    for part in range(4):
                        units.append(lambda part=part, vu=vus[a]: vu(part))
                        units.append(lambda part=part, vu=vus[b]: vu(part))
            for h in heads:
                for w_sb, dst in ((wq_sb, qt_sb), (wk_sb, kt_sb)):
                    st = {}

                    def qku(part, h=h, w_sb=w_sb, dst=dst, st=st, qs=qs,
                            qc=qc):
                        if part == 0:
                            st['ps'] = ps_qk.tile([P, QC], f32, tag="qk",
                                                  name=f"psqk{qc}_{h}")
                        ps = st['ps']
                        for di in range(part * 4, part * 4 + 4):
                            mm(ps,
                               w_sb[:, di * E + h * P: di * E + (h + 1) * P],
                               xts(qc, di),
                               start=(di == 0), stop=(di == NDT - 1))
                        if part == 3:
                            # RoPE: ro = ps*cos + shuffle_halves(ps)*sin_signed
                            tmp = evp.tile([P, QC], f32, tag="tmp")
                            nc.vector.tensor_mul(tmp[0:HF, :], ps[HF:P, :],
                                                 sin_sb[0:HF, qs])
                            nc.vector.tensor_mul(tmp[HF:P, :], ps[0:HF, :],
                                                 sin_sb[HF:P, qs])
                            ro2 = evp.tile([P, QC], f32, tag="ro2")
                            nc.vector.tensor_mul(ro2, ps, cos_sb[:, qs])
                            nc.vector.tensor_add(
                                dst[:, h * S + qc * QC: h * S + (qc + 1) * QC],
                                ro2, tmp)
                    for part in range(4):
                        units.append(lambda part=part, qku=qku: qku(part))
            return units

        # ---- B-stream: scores -> exp -> AV / den for one q-chunk ----
        pend = [None]

        def finalize():
            h, qc, ps_avt_t, den_sb = pend[0]
            rec = smp.tile([1, QC], f32, tag="rec")
            nc.vector.reciprocal_approx_fast(out=rec, in_=den_sb)
            bc = smp.tile([P, QC], f32, tag="bc")
            nc.gpsimd.partition_broadcast(bc, rec)
            nc.vector.tensor_mul(
                avt[:, h * S + qc * QC: h * S + (qc + 1) * QC], ps_avt_t, bc)
            pend[0] = None

        def b_steps(qc):
            steps = []
            nkt = JB * (qc + 1)
            for h in range(HPC):
                st = {}
                for ki in range(nkt + LAG):
                    def step(h=h, ki=ki, qc=qc, nkt=nkt, st=st):
                        qw = h * S + qc * QC
                        if ki < nkt:
                            j = ki - JB * qc
                            lo = j * P if j >= 0 else 0
                            ps_s = ps_sc.tile([P, QC], f32, tag="sc",
                                              name=f"pss{qc}_{h}_{ki}")
                            mm(ps_s[:, lo:QC],
                               kt_sb[:, h * S + ki * P: h * S + (ki + 1) * P],
                               qt_sb[:, qw + lo: qw + QC],
                               start=True, stop=True)
                            nc.scalar.activation(
                                et[:, ki * QC + lo:(ki + 1) * QC],
                                ps_s[:, lo:QC], Exp, scale=scale)
                            if j >= 0:
                                # causal mask as 0/1 mul on et AFTER exp --
                                # off the score->exp chain; AV/den read et
                                # LAG steps later so the DVE has slack
                                ets_d = et[:, ki * QC + j * P:
                                           ki * QC + (j + 1) * P]
                                nc.vector.tensor_mul(ets_d, ets_d, tri_sb)
                        if ki == 2 and pend[0] is not None:
                            finalize()
                        if ki >= LAG:
                            kj = ki - LAG
                            jj = kj - JB * qc
                            lo_j = jj * P if jj >= 0 else 0
                            ets_j = et[:, kj * QC + lo_j:(kj + 1) * QC]
                            if kj == 0:
                                st['ps_avt'] = ps_av.tile(
                                    [P, QC], f32, tag="avt",
                                    name=f"psavt{qc}_{h}")
                                st['ps_den'] = ps_dn.tile(
                                    [1, QC], f32, tag="den",
                                    name=f"psden{qc}_{h}")
                            mm(st['ps_avt'][:, lo_j:QC],
                               v_sb[:, kj * E + h * P: kj * E + (h + 1) * P],
                               ets_j,
                               start=(kj == 0), stop=(kj == nkt - 1))
                            mm(st['ps_den'][:, lo_j:QC], ones_bf, ets_j,
                               start=(kj == 0), stop=(kj == nkt - 1))
                            if kj == nkt - 1:
                                den_sb = smp.tile([1, QC], f32, tag="den")
                                nc.vector.tensor_copy(den_sb, st['ps_den'])
                                pend[0] = (h, qc, st['ps_avt'], den_sb)
                    steps.append(step)
            return steps

        # ---- O-stream: output projection + store for one q-chunk ----
        NST = S // P

        def o_units(qc, half_cfg):
            # half_cfg: ((ev, alt) for units 0..7, (ev, alt) for units 8..15).
            # alt=True alternates psy between ps_y and the ps_qk pool (free of
            # A-units in window 3 / drain) for true double-buffering.
            units = []
            for sl in range(QC // P):
                si = qc * (QC // P) + sl
                ev, alt = half_cfg[0] if sl < 2 else half_cfg[1]
                st = {}

                def ou(oc, si=si, st=st, ev=ev, alt=alt):
                    if oc == 0:
                        st['y'] = yp.tile([P, D], bf16, tag="y",
                                          name=f"ysb{si}")
                    pool = ps_qk if (alt and (si * NOC + oc) % 2) else ps_y
                    psy = pool.tile([P, QC], f32,
                                    tag="qk" if pool is ps_qk else "py",
                                    name=f"psy{si}_{oc}")
                    for h in range(HPC):
                        mm(psy,
                           avt[:, h * S + si * P: h * S + (si + 1) * P],
                           wo_sb[:, h * D + oc * QC: h * D + (oc + 1) * QC],
                           start=(h == 0), stop=(h == HPC - 1))
                    ysl = st['y'][:, oc * QC:(oc + 1) * QC]
                    if ev == 'split':
                        # both engines idle (drain): halve the evict latency
                        HQ = QC // 2
                        nc.scalar.copy(ysl[:, 0:HQ], psy[:, 0:HQ])
                        nc.vector.tensor_copy(ysl[:, HQ:QC], psy[:, HQ:QC])
                    elif ev == 'scalar':
                        nc.scalar.copy(ysl, psy)
                    else:
                        nc.vector.tensor_copy(ysl, psy)
                    if si == NST - 1 or ev == 'split':
                        # store per oc so the tail drains fast
                        nc.scalar.dma_start(
                            y[si * P:(si + 1) * P, oc * QC:(oc + 1) * QC],
                            ysl)
                    elif oc == NOC - 1:
                        nc.scalar.dma_start(y[si * P:(si + 1) * P, :],
                                            st['y'])
                for oc in range(NOC):
                    units.append(lambda oc=oc, ou=ou: ou(oc))
            return units

        def merge(a, b):
            out, ia, ib = [], 0, 0
            while ia < len(a) or ib < len(b):
                if ib >= len(b) or (ia < len(a) and
                                    ia * (len(b) or 1) <= ib * (len(a) or 1)):
                    out.append(a[ia]); ia += 1
                else:
                    out.append(b[ib]); ib += 1
            return out

        # ---- prologue: DMAs interleaved in first-use order, A(0) un-weaved --
        qs0 = slice(0, QC)
        for g in range(NDT // XG):
            load_w(wv_sb, wvT_t, g)
            x_g = xp.tile([P, XG * QC], bf16, tag="x", name=f"xg0_{g}")
            nc.sync.dma_start(x_g.rearrange("p (t s) -> p t s", t=XG),
                              xT_t[:, g * XG:(g + 1) * XG, qs0])
            xg_tiles[(0, g)] = x_g
        nc.sync.dma_start(cos_sb, cosT)
        nc.sync.dma_start(sin_sb, sinTs)
        nc.sync.dma_start(tri_sb, triD)
        for g in range(NDT // WG):
            load_w(wq_sb, wqT_t, g, queue=nc.sync)
        for g in range(NDT // WG):
            load_w(wk_sb, wkT_t, g)

        for u in a_units(0):
            u()

        # ---- main loop: window qc = B(qc) woven with A(qc+1) and O(<qc) ----
        o_prev = []
        for qc in range(NQC):
            units = []
            if qc < NQC - 1:
                dma_x(qc + 1)
                units = a_units(qc + 1)
            if qc == 0:
                nc.scalar.dma_start(
                    wo_sb.rearrange("p (t o) -> p t o", t=HPC), woT_t)
            ou = list(o_prev)
            o_prev = []
            if qc >= 1:
                if qc == 1:
                    cfg = (('scalar', False), ('scalar', False))
                elif qc == 2:
                    cfg = (('scalar', False), ('vector', True))
                else:
                    cfg = (('vector', True), ('vector', True))
                full = o_units(qc - 1, cfg)
                if qc < NQC - 1:
                    ou += full[:8]
                    o_prev = full[8:]
                else:
                    ou += full
            units = merge(units, ou)
            steps = b_steps(qc)
            n_steps, n_units = len(steps), len(units)
            emitted = 0
            # in the last window, hold back a few units as PE padding over
            # the final den/normalize chain's latency
            cap = n_units - 4 if qc == NQC - 1 else n_units
            for i, s in enumerate(steps):
                s()
                if i >= 3 and n_units:
                    tgt = min(cap, (n_units * (i - 2)) // max(1, n_steps - 3))
                    while emitted < tgt:
                        units[emitted]()
                        emitted += 1
            if qc == NQC - 1 and pend[0] is not None:
                # final den/normalize chain first so it runs concurrently
                # with the reserved padding units' matmuls
                finalize()
            while emitted < n_units:
                units[emitted]()
                emitted += 1
        if pend[0] is not None:
            finalize()
        for u in o_units(NQC - 1, (('split', True), ('split', True))):
            u()

    nc.compile()
    return nc


# ---------------------------------------------------------------------------
# Legacy builder (non-causal modes) -- unchanged from the shipped baseline.
# ---------------------------------------------------------------------------

def build_program_legacy(S=S, D=D, HPC=HPC, mode="none"):
    """Per-core Bass/Tile program for mode "none" | "general" (baseline)."""
    _ensure_paths()
    import concourse.mybir as mybir
    import concourse.tile as tile
    from concourse import bacc

    f32 = mybir.dt.float32
    f32r = mybir.dt.float32r
    Exp = mybir.ActivationFunctionType.Exp

    E = HPC * P
    NDT = D // P
    NQC = S // QC
    NKT = S // P
    NST = S // P
    NOC = D // QC
    JB = QC // P
    HF = P // 2
    scale = 1.0 / float(np.sqrt(DH))

    nc = bacc.Bacc("TRN2", target_bir_lowering=False, debug=False,
                   num_devices=N_CORES)

    xT = nc.dram_tensor("xT", [D, S], f32r, kind="ExternalInput").ap()
    wqT = nc.dram_tensor("wqT", [D, E], f32r, kind="ExternalInput").ap()
    wkT = nc.dram_tensor("wkT", [D, E], f32r, kind="ExternalInput").ap()
    wvT = nc.dram_tensor("wvT", [D, E], f32r, kind="ExternalInput").ap()
    woT = nc.dram_tensor("woT", [E, D], f32r, kind="ExternalInput").ap()
    cosT = nc.dram_tensor("cosT", [P, S], f32, kind="ExternalInput").ap()
    sinTs = nc.dram_tensor("sinTs", [P, S], f32, kind="ExternalInput").ap()
    if mode == "general":
        maskT = nc.dram_tensor("maskT", [S, S], f32, kind="ExternalInput").ap()
    y = nc.dram_tensor("y", [S, D], f32, kind="ExternalOutput").ap()
    qt_s = [nc.dram_tensor(f"qt_s{h}", [P, S], f32r).ap()
            for h in range(HPC)]
    kt_s = [nc.dram_tensor(f"kt_s{h}", [P, S], f32r).ap()
            for h in range(HPC)]

    xT_t = xT.rearrange("(t p) s -> p t s", p=P)
    wqT_t = wqT.rearrange("(t p) e -> p t e", p=P)
    wkT_t = wkT.rearrange("(t p) e -> p t e", p=P)
    wvT_t = wvT.rearrange("(t p) e -> p t e", p=P)
    woT_t = woT.rearrange("(t p) o -> p t o", p=P)
    if mode == "general":
        maskT_t = maskT.rearrange("(t p) q -> p t q", p=P)

    def mm(ps, lhsT, rhs, start, stop):
        nc.tensor.matmul(ps, lhsT=lhsT, rhs=rhs, start=start, stop=stop)

    with tile.TileContext(nc) as tc:
      with tc.tile_pool(name="persist", bufs=1) as persist:
        v_sb = persist.tile([P, NST * E], f32r)
        ones_col = persist.tile([P, 1], f32r)
        ones_row = persist.tile([1, P], f32r)
        qt0_sb = persist.tile([P, S], f32r)
        kt0_sb = persist.tile([P, S], f32r)
        ones_col_f = persist.tile([P, 1], f32)
        ones_row_f = persist.tile([1, P], f32)
        nc.vector.memset(ones_col_f, 1.0)
        nc.vector.memset(ones_row_f, 1.0)
        nc.vector.tensor_copy(ones_col, ones_col_f)
        nc.vector.tensor_copy(ones_row, ones_row_f)

        with tc.tile_pool(name="wp", bufs=1) as wp, \
             tc.tile_pool(name="csp", bufs=2) as csp, \
             tc.tile_pool(name="xtp", bufs=5) as xtp, \
             tc.tile_pool(name="evp", bufs=2) as evp, \
             tc.tile_pool(name="ps_v", bufs=2, space="PSUM") as ps_v, \
             tc.tile_pool(name="ps_qk", bufs=3, space="PSUM") as ps_qk:
            wv_sb = wp.tile([P, NDT * E], f32r)
            wq_sb = wp.tile([P, NDT * E], f32r)
            wk_sb = wp.tile([P, NDT * E], f32r)
            WG = 4

            def load_w(w_sb, w_t, g, t0=0, tn=None):
                t0 = g * WG + t0
                tn = tn if tn is not None else WG
                nc.sync.dma_start(
                    w_sb[:, t0 * E:(t0 + tn) * E].rearrange(
                        "p (t e) -> p t e", t=tn),
                    w_t[:, t0:t0 + tn])
            load_w(wv_sb, wvT_t, 0, 0, 1)
            load_w(wv_sb, wvT_t, 0, 1, 3)
            XG = 4
            for qc in range(NQC):
                qs = slice(qc * QC, (qc + 1) * QC)
                cos_t = csp.tile([P, QC], f32, tag="cos")
                sin_t = csp.tile([P, QC], f32, tag="sin")
                xg = []
                xts = []
                for g in range(NDT // XG):
                    x_g = xtp.tile([P, XG * QC], f32r, tag="xt")
                    if qc == 0 and g == 0:
                        nc.sync.dma_start(x_g[:, 0:QC], xT_t[:, 0, qs])
                        nc.sync.dma_start(
                            x_g[:, QC:XG * QC].rearrange(
                                "p (t s) -> p t s", t=XG - 1),
                            xT_t[:, 1:XG, qs])
                    else:
                        nc.sync.dma_start(
                            x_g.rearrange("p (t s) -> p t s", t=XG),
                            xT_t[:, g * XG:(g + 1) * XG, qs])
                    xg.append(x_g)
                    if qc == 0 and g < NDT // WG - 1:
                        load_w(wv_sb, wvT_t, g + 1)
                    xts += [x_g[:, j * QC:(j + 1) * QC] for j in range(XG)]
                nc.sync.dma_start(cos_t, cosT[:, qs])
                nc.sync.dma_start(sin_t, sinTs[:, qs])
                if qc == 0:
                    for g in range(NDT // WG):
                        load_w(wq_sb, wqT_t, g)
                        load_w(wk_sb, wkT_t, g)
                for sl in range(QC // P):
                    si = qc * (QC // P) + sl
                    psv = ps_v.tile([P, E], f32, tag="psv")
                    for di in range(NDT):
                        mm(psv, xts[di][:, sl * P:(sl + 1) * P],
                           wv_sb[:, di * E:(di + 1) * E],
                           start=(di == 0), stop=(di == NDT - 1))
                    nc.vector.tensor_copy(v_sb[:, si * E:(si + 1) * E], psv)
                for h in range(HPC):
                    for w_sb, dst, sb0 in ((wq_sb, qt_s, qt0_sb),
                                           (wk_sb, kt_s, kt0_sb)):
                        ps = ps_qk.tile([P, QC], f32, tag="psqk")
                        for di in range(NDT):
                            mm(ps,
                               w_sb[:, di * E + h * P: di * E + (h + 1) * P],
                               xts[di],
                               start=(di == 0), stop=(di == NDT - 1))
                        tmp = evp.tile([P, QC], f32, tag="tmp")
                        nc.vector.tensor_mul(tmp[0:HF, :], ps[HF:P, :],
                                             sin_t[0:HF, :])
                        nc.vector.tensor_mul(tmp[HF:P, :], ps[0:HF, :],
                                             sin_t[HF:P, :])
                        ro2 = evp.tile([P, QC], f32, tag="ro2")
                        nc.vector.tensor_mul(ro2, ps, cos_t)
                        if h == 0:
                            nc.vector.tensor_add(sb0[:, qs], ro2, tmp)
                        else:
                            ro = evp.tile([P, QC], f32r, tag="ro")
                            nc.vector.tensor_add(ro, ro2, tmp)
                            nc.sync.dma_start(dst[h][:, qs], ro)

        with tc.tile_pool(name="bcp", bufs=1) as bcp:
            avt = bcp.tile([P, HPC * S], f32r)
            wo_sb = bcp.tile([P, HPC * D], f32r)
            nc.sync.dma_start(
                wo_sb.rearrange("p (t o) -> p t o", t=HPC), woT_t)

            with tc.tile_pool(name="qkp", bufs=2) as qkp, \
                 tc.tile_pool(name="etp", bufs=1) as etp, \
                 tc.tile_pool(name="mkp", bufs=2) as mkp, \
                 tc.tile_pool(name="smp", bufs=2) as smp, \
                 tc.tile_pool(name="ps_sc", bufs=4, space="PSUM") as ps_sc, \
                 tc.tile_pool(name="ps_av", bufs=2, space="PSUM") as ps_av, \
                 tc.tile_pool(name="ps_dn", bufs=1, space="PSUM") as ps_dn, \
                 tc.tile_pool(name="ps_bc", bufs=1, space="PSUM") as ps_bc:
                pending = None

                def finalize(fin):
                    h, qc, ps_avt, den_sb = fin
                    psb = ps_bc.tile([P, QC], f32, tag="bc")
                    mm(psb, ones_row, den_sb, start=True, stop=True)
                    bc_sb = smp.tile([P, QC], f32, tag="bcs")
                    nc.vector.reciprocal_approx_fast(out=bc_sb, in_=psb)
                    nc.vector.tensor_mul(
                        avt[:, h * S + qc * QC: h * S + (qc + 1) * QC],
                        ps_avt, bc_sb)

                for h in range(HPC):
                    if h == 0:
                        qt, kt = qt0_sb, kt0_sb
                    else:
                        qt = qkp.tile([P, S], f32r, tag="qt")
                        kt = qkp.tile([P, S], f32r, tag="kt")
                        nc.sync.dma_start(qt, qt_s[h])
                        nc.sync.dma_start(kt, kt_s[h])
                    for qc in range(NQC):
                        nkt = NKT
                        qs = slice(qc * QC, (qc + 1) * QC)
                        et = etp.tile([P, NKT * QC], f32r, tag="et")
                        ps_avt = ps_av.tile([P, QC], f32, tag="avt")
                        ps_den = ps_dn.tile([1, QC], f32, tag="den")
                        if mode == "general":
                            mk = mkp.tile([P, NKT * QC], f32, tag="mk")
                            nc.sync.dma_start(
                                mk.rearrange("p (t q) -> p t q", t=NKT),
                                maskT_t[:, :, qs])

                        LAG = 4 if nkt >= 4 else nkt
                        for ki in range(nkt + LAG):
                            if ki < nkt:
                                ps_s = ps_sc.tile([P, QC], f32, tag="sc")
                                mm(ps_s, kt[:, ki * P:(ki + 1) * P],
                                   qt[:, qs], start=True, stop=True)
                                ets = et[:, ki * QC:(ki + 1) * QC]
                                if mode == "general":
                                    nc.vector.tensor_add(
                                        ps_s, ps_s,
                                        mk[:, ki * QC:(ki + 1) * QC])
                                nc.scalar.activation(ets, ps_s, Exp,
                                                     scale=scale)
                            if ki == 5 and pending is not None:
                                finalize(pending)
                                pending = None
                            if ki >= LAG:
                                kj = ki - LAG
                                ets_j = et[:, kj * QC:(kj + 1) * QC]
                                mm(ps_den, ones_col, ets_j,
                                   start=(kj == 0), stop=(kj == nkt - 1))
                                mm(ps_avt,
                                   v_sb[:, kj * E + h * P:
                                        kj * E + (h + 1) * P],
                                   ets_j,
                                   start=(kj == 0), stop=(kj == nkt - 1))
                        den_sb = smp.tile([1, QC], f32r, tag="den")
                        nc.vector.tensor_copy(den_sb, ps_den)
                        pending = (h, qc, ps_avt, den_sb)
                if pending is not None:
                    finalize(pending)
                    pending = None

            with tc.tile_pool(name="yp", bufs=2) as yp, \
                 tc.tile_pool(name="ps_c", bufs=4, space="PSUM") as ps_c:
                for si in range(NST):
                    yt = yp.tile([P, D], f32, tag="yt")
                    for oc in range(NOC):
                        psy = ps_c.tile([P, QC], f32, tag="py")
                        for h in range(HPC):
                            mm(psy,
                               avt[:, h * S + si * P: h * S + (si + 1) * P],
                               wo_sb[:, h * D + oc * QC:
                                     h * D + (oc + 1) * QC],
                               start=(h == 0), stop=(h == HPC - 1))
                        if si == NST - 1 and oc % 2 == 1:
                            nc.vector.tensor_copy(
                                yt[:, oc * QC:(oc + 1) * QC], psy)
                        else:
                            nc.scalar.copy(yt[:, oc * QC:(oc + 1) * QC], psy)
                        if si == NST - 1:
                            nc.sync.dma_start(
                                y[si * P:(si + 1) * P,
                                  oc * QC:(oc + 1) * QC],
                                yt[:, oc * QC:(oc + 1) * QC])
                    if si < NST - 1:
                        nc.sync.dma_start(y[si * P:(si + 1) * P, :], yt)

    nc.compile()
    return nc


def round_f32r(a):
    """Round fp32 to the PE's fp32r (TF32-like, 11-bit mantissa) encoding."""
    u = np.ascontiguousarray(a, dtype=np.float32).view(np.uint32)
    u = ((u.astype(np.int64) + 0x800) & 0xFFFFF000).astype(np.uint32)
    return u.view(np.float32)


def _rope_tables(S_):
    inv_freq = 1.0 / (10000.0 ** (np.arange(0, DH, 2, dtype=np.float32) / DH))
    t = np.arange(S_, dtype=np.float32)
    freqs = np.outer(t, inv_freq)
    emb = np.concatenate([freqs, freqs], axis=-1)      # [S, dh]
    cosT = np.ascontiguousarray(np.cos(emb).T, dtype=np.float32)
    sinT = np.sin(emb).T.astype(np.float32)
    sinTs = np.concatenate([-sinT[:DH // 2], sinT[DH // 2:]], 0)
    return cosT, np.ascontiguousarray(sinTs, dtype=np.float32)


def host_inputs(x, attention_mask, wq, wk, wv, wo, mode):
    """Build the 8 per-core input maps from the full problem inputs."""
    S_ = x.shape[1]
    E = HPC * DH
    cosT, sinTs = _rope_tables(S_)

    if mode == "causal":
        import ml_dtypes
        bf = ml_dtypes.bfloat16
        pp = np.arange(P)[:, None]
        cc = np.arange(P)[None, :]
        tri = np.where(pp <= cc, np.float32(1.0),
                       np.float32(0.0)).astype(bf)
        in_maps = []
        for core in range(N_CORES):
            b, g = divmod(core, GROUPS)
            r = slice(g * E, (g + 1) * E)
            in_maps.append({
                "xT": np.ascontiguousarray(x[b].T).astype(bf),
                "wqT": np.ascontiguousarray(wq[r].T).astype(bf),
                "wkT": np.ascontiguousarray(wk[r].T).astype(bf),
                "wvT": np.ascontiguousarray(wv[r].T).astype(bf),
                "woT": np.ascontiguousarray(wo[:, r].T).astype(bf),
                "cosT": cosT.astype(bf),
                "sinTs": sinTs.astype(bf),
                "triD": tri,
            })
        return in_maps

    extra = {}
    if mode == "general":
        extra["maskT"] = np.ascontiguousarray(
            attention_mask[0, 0].T, dtype=np.float32)
    in_maps = []
    for core in range(N_CORES):
        b, g = divmod(core, GROUPS)
        r = slice(g * E, (g + 1) * E)
        in_maps.append({
            "xT": round_f32r(x[b].T),
            "wqT": round_f32r(wq[r].T),
            "wkT": round_f32r(wk[r].T),
            "wvT": round_f32r(wv[r].T),
            "woT": round_f32r(wo[:, r].T),
            "cosT": cosT,
            "sinTs": sinTs,
            **extra,
        })
    return in_maps


def detect_mode(attention_mask):
    m = attention_mask[0, 0]
    if not np.any(m):
        return "none"
    S_ = m.shape[0]
    causal = np.triu(np.full((S_, S_), -1e9, dtype=np.float32), k=1)
    if np.array_equal(m, causal):
        return "causal"
    return "general"


def kernel(**inputs):
    _ensure_paths()
    from concourse.bass_utils import run_bass_kernel_spmd

    x = np.asarray(inputs["x"], dtype=np.float32)
    mask = np.asarray(inputs["attention_mask"], dtype=np.float32)
    wq = np.asarray(inputs["wq"], dtype=np.float32)
    wk = np.asarray(inputs["wk"], dtype=np.float32)
    wv = np.asarray(inputs["wv"], dtype=np.float32)
    wo = np.asarray(inputs["wo"], dtype=np.float32)

    mode = detect_mode(mask)
    if mode not in _CACHE:
        if mode == "causal":
            _CACHE[mode] = build_fused_causal()
        else:
            _CACHE[mode] = build_program_legacy(mode=mode)
    nc = _CACHE[mode]

    in_maps = host_inputs(x, mask, wq, wk, wv, wo, mode)
    res = run_bass_kernel_spmd(nc, in_maps, core_ids=list(range(N_CORES)))

    out = np.zeros((B, S, D), dtype=np.float32)
    for core in range(N_CORES):
        b = core // GROUPS
        out[b] += np.asarray(res.results[core]["y"], dtype=np.float32)
    return out
